# revision 67
# baseline (speedup 1.0000x reference)
"""
Trainium2 Bass kernel for batched cross-attention:
  context[b] = softmax(q[b] @ tokens[b].T / sqrt(d)) @ tokens[b]
with x_latent (tokens) [16, 4096, 768] f32, prompts_latent (q) [16, 64, 768] f32.

Sharding: data-parallel over batch - 16 batches / 8 cores = 2 per core.

Design (v7):
  - Single-phase pipeline: group g's transposes AND its mm1 run in the same
    phase, with mm1 emitted j-sliced (N=128 chunks) one j-slice behind the
    transposes.  pT+mm2 of group g-1 are slotted between.  Every PE
    instruction's off-engine dependencies (PSUM->SBUF copies, exp) complete
    well before the strict-FIFO tensor queue reaches it, and the steady
    trickle of normal-mode matmuls keeps the HAM clock-gate at 8/8.
  - The transposes' 128-col LDWEIGHTS hide under the mm1/mm2 streams on the
    weight port.
  - The trailing G1 groups' [d, n] tiles are shipped pre-transposed and
    processed last: the kernel tail is a short mm1->exp->pT->mm2 chain,
    split in column halves to overlap exp with pT/mm2.
  - All PSUM->SBUF transpose copies ride the vector engine (fast, 530ns);
    scalar only does exp/accum plus the o_b normalize.

All operands bf16, accumulation f32.
"""

import os
import sys

import numpy as np

for _p in ("/opt/trn_rl_repo", "/root/.axon_site/_ro/trn_rl_repo"):
    if os.path.isdir(_p) and _p not in sys.path:
        sys.path.append(_p)

import ml_dtypes
from contextlib import ExitStack

import concourse.bass as bass
import concourse.mybir as mybir
import concourse.tile as tile
from concourse import bacc
from concourse.bass_utils import run_bass_kernel_spmd
from concourse.masks import make_identity

BF16 = ml_dtypes.bfloat16

N_CORES = 8
B_TOTAL = 16
BPC = B_TOTAL // N_CORES  # batches per core
N = 4096  # tokens
D = 768   # latent dim
P = 64    # prompts
DC = D // 128   # d-chunks of 128 (contraction tiles for mm1)
G = N // 512    # groups of 512 token-columns
NPAIR = G // 2  # tn is stored in pairs of groups
G1 = 2          # trailing groups whose [d, n] tiles come via DMA
NH = 1          # additional groups whose first 3 c-chunks ship pre-transposed
NT = N // 128
SCALE = float(D) ** -0.5

_cached_nc = None


def build_bass_program() -> bass.Bass:
    nc = bacc.Bacc("TRN2", target_bir_lowering=False, debug=False)
    qt = nc.declare_dram_parameter("qt", [128, BPC, DC, P], mybir.dt.bfloat16, isOutput=False)
    tn = nc.declare_dram_parameter("tn", [NPAIR, 128, 8, BPC, D], mybir.dt.bfloat16, isOutput=False)
    if G1 > 0:
        tt = nc.declare_dram_parameter("tt", [G1, 128, BPC, DC, 512], mybir.dt.bfloat16, isOutput=False)
    # groups NS-NH..NS-1 are "half shipped": their first HC c-chunks come
    # pre-transposed
    HC = 3
    NH = 1
    tth = nc.declare_dram_parameter("tth", [NH, 128, BPC, HC, 512], mybir.dt.bfloat16, isOutput=False)
    out = nc.declare_dram_parameter("out", [BPC, P, D], mybir.dt.bfloat16, isOutput=True)

    NS = G - G1  # groups 0..NS-1 are transposed on-chip

    with tile.TileContext(nc) as tc, ExitStack() as ctx:
        singles = ctx.enter_context(tc.tile_pool(name="singles", bufs=3))
        tn_pool = ctx.enter_context(tc.tile_pool(name="tn", bufs=4))
        tts_pool = ctx.enter_context(tc.tile_pool(name="tts", bufs=2))
        ttd_pool = ctx.enter_context(tc.tile_pool(name="ttd", bufs=max(G1, 1)))
        p_pool = ctx.enter_context(tc.tile_pool(name="pexp", bufs=3))
        pt_pool = ctx.enter_context(tc.tile_pool(name="ptT", bufs=2))
        o_pool = ctx.enter_context(tc.tile_pool(name="osb", bufs=1))
        sc_pool = ctx.enter_context(tc.tile_pool(name="scal", bufs=2))

        psum_s = ctx.enter_context(tc.tile_pool(name="psum_s", bufs=1, space="PSUM"))
        psum_tr = ctx.enter_context(tc.tile_pool(name="psum_tr", bufs=4, space="PSUM"))
        # one bank for the shipped groups' h0 score halves (pT tiles ride the
        # tr pool instead) so shipped mm1 never waits on the previous exp
        psum_sh = ctx.enter_context(tc.tile_pool(name="psum_sh", bufs=1, space="PSUM"))
        psum_o = ctx.enter_context(tc.tile_pool(name="psum_o", bufs=1, space="PSUM"))

        ident = singles.tile([128, 128], mybir.dt.bfloat16)

        # HAM warm-up: the clock-gate flip needs one full 3.4us window of
        # continuous normal-mode matmuls.  A DVE memset provides the dummy
        # operand ~1.3us before the gpsimd-built identity would be ready, so
        # the warmup (and hence the flip) starts as early as possible.
        dummy = singles.tile([128, 128], mybir.dt.bfloat16, name="dummy")
        nc.vector.memset(dummy, 0.0)
        warm = psum_s.tile([128, 512], mybir.dt.float32, name="s_ps")
        for w in range(34):
            nc.tensor.matmul(
                warm[:, (w % 4) * 128:(w % 4 + 1) * 128],
                lhsT=dummy,
                rhs=dummy,
                start=True,
                stop=True,
            )
        make_identity(nc, ident)

        # softmax denominators: col g for tr-groups (full exp accum), two
        # cols per shipped group's halves, last col for the final group's
        # DVE-reduced sum
        SUMW = NS + 2 * (G - 1 - NS) + 1
        sums_t = singles.tile([128, SUMW], mybir.dt.float32, name="sums_t")

        o_ab = [None]
        tn_tiles = {}   # pair -> tile [128, 8, BPC, D]
        ttd_tiles = {}  # g -> tile [128, BPC, DC, 512]
        tts_tiles = {}  # g -> tile [128, BPC, DC, 512] (on-chip transposed)
        p_tiles = {}    # g -> p_sb tile
        pt_tiles = {}   # g -> pt_sb tile
        mm1_psum = {}   # g -> s_ps tile

        def load_pair(p, eng, split=0):
            t = tn_pool.tile([128, 8, BPC, D], mybir.dt.bfloat16, name="tn_p")
            if split:
                step = 8 // split
                for h in range(split):
                    eng.dma_start(
                        out=t[:, h * step:(h + 1) * step],
                        in_=tn[p, :, h * step:(h + 1) * step],
                    )
            else:
                eng.dma_start(out=t, in_=tn[p])
            tn_tiles[p] = t

        def load_ttd(g, eng):
            gg = g - (G - G1)
            t = ttd_pool.tile([128, BPC, DC, 512], mybir.dt.bfloat16, name="tt_d")
            eng.dma_start(out=t, in_=tt[gg])
            ttd_tiles[g] = t

        def tr_slice(g, b, j, c0=0):
            """Transpose c-chunks c0..DC-1 of (group g, batch b, j-slice j):
            PE transposes into one PSUM tile, then one DVE copy into tts(g)."""
            if g not in tts_tiles:
                tts_tiles[g] = tts_pool.tile(
                    [128, BPC, DC, 512], mybir.dt.bfloat16, name="tts"
                )
            tn_p = tn_tiles[g // 2]
            jj0 = (g % 2) * 4
            tr = psum_tr.tile([128, DC, 128], mybir.dt.bfloat16, name="tr_ps")
            for c in range(c0, DC):
                nc.tensor.transpose(
                    tr[:, c - c0, :],
                    tn_p[:, jj0 + j, b, c * 128:(c + 1) * 128],
                    ident,
                )
            tr = tr[:, 0:DC - c0, :]
            dst = tts_tiles[g][:, b, c0:DC, j * 128:(j + 1) * 128]
            # copies split across DVE and ACT so the PSUM pool drains fast
            # enough for the 8-slice transpose burst; the last two slices'
            # copies stay on the faster DVE so mm1 never waits
            if (b, j) in ((1, 0), (1, 1), (1, 2)):
                nc.scalar.copy(dst, tr)
            else:
                nc.vector.tensor_copy(dst, tr)

        def mm1_chunk(g, c):
            """mm1 c-chunk, N=512, both batches col-tiled."""
            if g not in mm1_psum:
                mm1_psum[g] = psum_s.tile([128, 512], mybir.dt.float32, name="s_ps")
            s_ps = mm1_psum[g]
            src = ttd_tiles.get(g)
            if src is None:
                if NS - NH <= g < NS and c < HC:
                    for b in range(BPC):
                        nc.tensor.matmul(
                            s_ps[b * P:(b + 1) * P, :],
                            lhsT=qt_t[:, b, c, :],
                            rhs=tth_t[:, g - (NS - NH), b, c, :],
                            start=(c == 0),
                            stop=(c == DC - 1),
                            tile_position=(0, b * P),
                        )
                    return
                src = tts_tiles[g]
            for b in range(BPC):
                nc.tensor.matmul(
                    s_ps[b * P:(b + 1) * P, :],
                    lhsT=qt_t[:, b, c, :],
                    rhs=src[:, b, c, :],
                    start=(c == 0),
                    stop=(c == DC - 1),
                    tile_position=(0, b * P),
                )



        def mm1_half(g, c, h):
            """mm1 c-chunk restricted to n-column half h (N=256); used for
            shipped groups so exp of half h fires as soon as h stops.  h0
            targets the dedicated psum_sh bank (no buffer-reuse wait on the
            previous group's exp); h1 targets the regular s_ps bank."""
            if (g, h) not in mm1_psum:
                if h == 0:
                    mm1_psum[(g, h)] = psum_sh.tile(
                        [128, 256], mybir.dt.float32, name="s_sh"
                    )
                else:
                    mm1_psum[(g, h)] = psum_s.tile(
                        [128, 512], mybir.dt.float32, name="s_ps"
                    )
            t_ps = mm1_psum[(g, h)]
            dst = t_ps if h == 0 else t_ps[:, 256:512]
            src = ttd_tiles.get(g)
            if src is None:
                src = tts_tiles[g]
            for b in range(BPC):
                nc.tensor.matmul(
                    dst[b * P:(b + 1) * P, :],
                    lhsT=qt_t[:, b, c, :],
                    rhs=src[:, b, c, h * 256:(h + 1) * 256],
                    start=(c == 0),
                    stop=(c == DC - 1),
                    tile_position=(0, b * P),
                )

        def exp_full(g):
            s_ps = mm1_psum.pop(g)
            p_sb = p_pool.tile([128, 512], mybir.dt.bfloat16, name="p_sb")
            nc.scalar.activation(
                out=p_sb,
                in_=s_ps,
                func=mybir.ActivationFunctionType.Exp,
                scale=SCALE,
                accum_out=sums_t[:, g:g + 1],
            )
            p_tiles[g] = p_sb

        def exp_half(g, h):
            # per-half accum for shipped groups except the last (DVE reduce
            # in the tail instead)
            t_ps = mm1_psum[(g, h)]
            src = t_ps if h == 0 else t_ps[:, 256:512]
            if g not in p_tiles:
                p_tiles[g] = p_pool.tile([128, 512], mybir.dt.bfloat16, name="p_sb")
            kw = {}
            if g < G - 1:
                col = NS + 2 * (g - NS) + h
                kw["accum_out"] = sums_t[:, col:col + 1]
            nc.scalar.activation(
                out=p_tiles[g][:, h * 256:(h + 1) * 256],
                in_=src,
                func=mybir.ActivationFunctionType.Exp,
                scale=SCALE,
                **kw,
            )

        def p_transpose(g, js=(0, 1, 2, 3)):
            p_sb = p_tiles[g]
            if g not in pt_tiles:
                pt_tiles[g] = pt_pool.tile([128, 4, 128], mybir.dt.bfloat16, name="pt_sb")
            pt_ps = psum_tr.tile([128, DC, 128], mybir.dt.bfloat16, name="tr_ps")
            for k, j in enumerate(js):
                nc.tensor.transpose(
                    pt_ps[:, k, :], p_sb[:, j * 128:(j + 1) * 128], ident
                )
            nc.vector.tensor_copy(
                pt_tiles[g][:, js[0]:js[0] + len(js), :], pt_ps[:, 0:len(js), :]
            )

        def mm2_j(g, j):
            if o_ab[0] is None:
                o_a = psum_o.tile([128, 512], mybir.dt.float32, tag="o_a")
                o_b = psum_o.tile([128, 256], mybir.dt.float32, tag="o_b")
                o_ab[0] = (o_a, o_b)
            o_a, o_b = o_ab[0]
            tn_p = tn_tiles[g // 2]
            jj0 = (g % 2) * 4
            pt_sb = pt_tiles[g]
            nt = g * 4 + j
            for b in range(BPC):
                nc.tensor.matmul(
                    o_a[b * P:(b + 1) * P, :],
                    lhsT=pt_sb[:, j, b * P:(b + 1) * P],
                    rhs=tn_p[:, jj0 + j, b, 0:512],
                    start=(nt == 0),
                    stop=(nt == NT - 1),
                    tile_position=(0, b * P),
                )
                nc.tensor.matmul(
                    o_b[b * P:(b + 1) * P, :],
                    lhsT=pt_sb[:, j, b * P:(b + 1) * P],
                    rhs=tn_p[:, jj0 + j, b, 512:768],
                    start=(nt == 0),
                    stop=(nt == NT - 1),
                    tile_position=(0, b * P),
                )

        def finish():
            # sums_t[:, G-1] was filled by a DVE reduce over p(G-1)
            tot = sc_pool.tile([128, 1], mybir.dt.float32, name="tot")
            nc.vector.reduce_sum(tot, sums_t, axis=mybir.AxisListType.X)
            rec = sc_pool.tile([128, 1], mybir.dt.float32, name="rec")
            nc.vector.reciprocal(rec, tot)
            o_a, o_b = o_ab[0]
            o_sb = o_pool.tile([128, D], mybir.dt.bfloat16, name="o_sb")
            nc.vector.tensor_scalar_mul(o_sb[:, 0:512], o_a, rec)
            nc.scalar.mul(o_sb[:, 512:768], o_b, rec)
            nc.sync.dma_start(out=out[0], in_=o_sb[0:P, :])
            nc.scalar.dma_start(out=out[1], in_=o_sb[P:2 * P, :])

        # ---- DMA schedule ----
        # The 16-SDMA-engine pipe moves ~424 GB/s with a straggler engine a
        # few us behind; order transfers so each consumer runs just after its
        # semaphore: tt6 before pair3h1 (mm1(6) precedes mm2(6)), tt7 between
        # the pair3 halves.
        qt_t = singles.tile([128, BPC, DC, P], mybir.dt.bfloat16, name="qt_t")
        nc.scalar.dma_start(out=qt_t, in_=qt[:])
        load_pair(0, nc.sync, split=4)
        load_pair(1, nc.sync, split=2)
        load_pair(2, nc.sync, split=2)
        tth_t = singles.tile([128, NH, BPC, HC, 512], mybir.dt.bfloat16, name="tth_t")
        for hh in range(NH):
            nc.sync.dma_start(out=tth_t[:, hh], in_=tth[hh])
        load_ttd(G - 2, nc.sync)
        p3 = tn_pool.tile([128, 8, BPC, D], mybir.dt.bfloat16, name="tn_p")
        tn_tiles[3] = p3
        nc.sync.dma_start(out=p3[:, 0:4], in_=tn[3, :, 0:4])
        load_ttd(G - 1, nc.sync)
        nc.sync.dma_start(out=p3[:, 4:8], in_=tn[3, :, 4:8])

        # ---- phase-pipelined emission ----
        # Phase t (t < NS): transposes of group t, then mm1(t) (N=512
        # c-chunks), with pT(t-1)+mm2(t-1) slotted so every PE instruction's
        # off-engine producer (DVE copy / ACT exp) completes well before the
        # strict-FIFO tensor queue reaches it; exp(t) at the end.
        # Phase t in [NS, G): shipped group t - mm1 chunks first (no deps
        # beyond DMA), then pT/mm2 of t-1.
        # Phase G: split tail of group G-1, then finish.
        # Each steady phase is ONE transpose-mode mega-burst (48 token
        # transposes + the 4 pT transposes of the previous group) followed by
        # ONE normal-mode burst (mm2 of g-1, mm1 of g, exp) - mode switches
        # between transpose-mode and normal matmuls cost ~95ns each, so they
        # are minimized to two per phase.
        for t in range(G):
            g_pre = t - 1
            if t < NS:
                tc0 = HC if NS - NH <= t < NS else 0
                tr_slice(t, 0, 0, tc0)
                tr_slice(t, 1, 0, tc0)
                tr_slice(t, 0, 1, tc0)
                tr_slice(t, 1, 1, tc0)
                tr_slice(t, 0, 2, tc0)
                tr_slice(t, 1, 2, tc0)
                if g_pre >= 0:
                    p_transpose(g_pre)
                tr_slice(t, 0, 3, tc0)
                tr_slice(t, 1, 3, tc0)
                if g_pre >= 0:
                    mm2_j(g_pre, 0)
                    mm2_j(g_pre, 1)
                    mm2_j(g_pre, 2)
                    mm2_j(g_pre, 3)
                for c in range(DC):
                    mm1_chunk(t, c)
                exp_full(t)
            elif t < G - 1:
                # shipped group: mm1 in column halves so each exp half fires
                # as soon as its accumulation stops
                for c in range(DC):
                    mm1_half(t, c, 0)
                p_transpose(g_pre)
                exp_half(t, 0)
                for c in range(DC):
                    mm1_half(t, c, 1)
                mm2_j(g_pre, 0)
                mm2_j(g_pre, 1)
                mm2_j(g_pre, 2)
                exp_half(t, 1)
                mm2_j(g_pre, 3)
            else:
                # last group: its pT halves are prefetched into this phase so
                # the tail is a dense mm2 chain with no copy waits
                for c in range(DC):
                    mm1_half(t, c, 0)
                p_transpose(g_pre)
                exp_half(t, 0)
                for c in range(DC):
                    mm1_half(t, c, 1)
                mm2_j(g_pre, 0)
                p_transpose(t, js=(0, 1))
                mm2_j(g_pre, 1)
                mm2_j(g_pre, 2)
                exp_half(t, 1)
                mm2_j(g_pre, 3)

        # tail of the last group: pT(0,1) already copied; only pT(2,3) left
        g = G - 1
        mm2_j(g, 0)
        p_transpose(g, js=(2, 3))
        mm2_j(g, 1)
        # last group's row-sum: after the pt copies so the DVE queue never
        # delays mm2's pt dependency
        nc.vector.reduce_sum(
            sums_t[:, SUMW - 1:SUMW], p_tiles[g], axis=mybir.AxisListType.X
        )
        mm2_j(g, 2)
        mm2_j(g, 3)
        finish()

    nc.compile()
    return nc


def _get_nc() -> bass.Bass:
    global _cached_nc
    if _cached_nc is None:
        _cached_nc = build_bass_program()
    return _cached_nc


def _make_in_maps(x_latent: np.ndarray, prompts_latent: np.ndarray):
    x8 = np.ascontiguousarray(x_latent.astype(BF16)).reshape(N_CORES, BPC, N, D)
    q8 = prompts_latent.astype(BF16).reshape(N_CORES, BPC, P, D)
    # tn: [core, NPAIR, 128, 8, BPC, D] - j-major so split loads slice
    # contiguous per-partition runs
    tn_sw = np.ascontiguousarray(
        x8.reshape(N_CORES, BPC, NPAIR, 8, 128, D).transpose(0, 2, 4, 3, 1, 5)
    )
    # qt: [core, 128, BPC, DC, P]
    qt_sw = np.ascontiguousarray(
        q8.transpose(0, 1, 3, 2).reshape(N_CORES, BPC, DC, 128, P).transpose(0, 3, 1, 2, 4)
    )
    maps = []
    ttf = x8.transpose(0, 1, 3, 2)                      # [core, b, D, N]
    arr = ttf.reshape(N_CORES, BPC, DC, 128, G, 512)
    if G1 > 0:
        # tt: [core, G1, 128, BPC, DC, 512]
        tt_sw = np.ascontiguousarray(
            arr[:, :, :, :, G - G1:, :].transpose(0, 4, 3, 1, 2, 5)
        )
    # tth: first 3 c-chunks of groups G-G1-2, G-G1-1: [core, 2, 128, BPC, 3, 512]
    tth_sw = np.ascontiguousarray(
        arr[:, :, 0:3, :, G - G1 - NH:G - G1, :].transpose(0, 4, 3, 1, 2, 5)
    )
    for c in range(N_CORES):
        m = {"qt": qt_sw[c], "tn": tn_sw[c], "tth": tth_sw[c]}
        if G1 > 0:
            m["tt"] = tt_sw[c]
        maps.append(m)
    return maps


def run(x_latent: np.ndarray, prompts_latent: np.ndarray, trace: bool = False):
    """Run on all 8 cores; returns (output [16, 64, 768] f32, BassKernelResults)."""
    nc = _get_nc()
    in_maps = _make_in_maps(np.asarray(x_latent), np.asarray(prompts_latent))
    res = run_bass_kernel_spmd(nc, in_maps, list(range(N_CORES)), trace=trace)
    out = np.concatenate(
        [np.asarray(r["out"]).astype(np.float32) for r in res.results], axis=0
    )
    return out, res


def kernel(x_latent: np.ndarray, prompts_latent: np.ndarray) -> np.ndarray:
    out, _ = run(x_latent, prompts_latent, trace=False)
    return out


# revision 71
# speedup vs baseline: 1.0947x; 1.0947x over previous
"""
Trainium2 Bass kernel for batched cross-attention:
  context[b] = softmax(q[b] @ tokens[b].T / sqrt(d)) @ tokens[b]
with x_latent (tokens) [16, 4096, 768] f32, prompts_latent (q) [16, 64, 768] f32.

Sharding: data-parallel over batch - 16 batches / 8 cores = 2 per core.

Design (v7):
  - Single-phase pipeline: group g's transposes AND its mm1 run in the same
    phase, with mm1 emitted j-sliced (N=128 chunks) one j-slice behind the
    transposes.  pT+mm2 of group g-1 are slotted between.  Every PE
    instruction's off-engine dependencies (PSUM->SBUF copies, exp) complete
    well before the strict-FIFO tensor queue reaches it, and the steady
    trickle of normal-mode matmuls keeps the HAM clock-gate at 8/8.
  - The transposes' 128-col LDWEIGHTS hide under the mm1/mm2 streams on the
    weight port.
  - The trailing G1 groups' [d, n] tiles are shipped pre-transposed and
    processed last: the kernel tail is a short mm1->exp->pT->mm2 chain,
    split in column halves to overlap exp with pT/mm2.
  - All PSUM->SBUF transpose copies ride the vector engine (fast, 530ns);
    scalar only does exp/accum plus the o_b normalize.

All operands bf16, accumulation f32.
"""

import os
import sys

import numpy as np

for _p in ("/opt/trn_rl_repo", "/root/.axon_site/_ro/trn_rl_repo"):
    if os.path.isdir(_p) and _p not in sys.path:
        sys.path.append(_p)

import ml_dtypes
from contextlib import ExitStack

import concourse.bass as bass
import concourse.mybir as mybir
import concourse.tile as tile
from concourse import bacc
from concourse.bass_utils import run_bass_kernel_spmd
from concourse.masks import make_identity

BF16 = ml_dtypes.bfloat16

N_CORES = 8
B_TOTAL = 16
BPC = B_TOTAL // N_CORES  # batches per core
N = 4096  # tokens
D = 768   # latent dim
P = 64    # prompts
DC = D // 128   # d-chunks of 128 (contraction tiles for mm1)
G = N // 512    # groups of 512 token-columns
NPAIR = G // 2  # tn is stored in pairs of groups
G1 = 2          # trailing groups whose [d, n] tiles come via DMA
NH = 1          # additional groups whose first 3 c-chunks ship pre-transposed
NT = N // 128
SCALE = float(D) ** -0.5

_cached_nc = None


def build_bass_program() -> bass.Bass:
    nc = bacc.Bacc("TRN2", target_bir_lowering=False, debug=False)
    qt = nc.declare_dram_parameter("qt", [128, BPC, DC, P], mybir.dt.bfloat16, isOutput=False)
    tn = nc.declare_dram_parameter("tn", [NPAIR, 128, 8, BPC, D], mybir.dt.bfloat16, isOutput=False)
    if G1 > 0:
        tt = nc.declare_dram_parameter("tt", [G1, 128, BPC, DC, 512], mybir.dt.bfloat16, isOutput=False)
    # groups NS-NH..NS-1 are "half shipped": their first HC c-chunks come
    # pre-transposed
    HC = 3
    NH = 1
    tth = nc.declare_dram_parameter("tth", [NH, 128, BPC, HC, 512], mybir.dt.bfloat16, isOutput=False)
    out = nc.declare_dram_parameter("out", [BPC, P, D], mybir.dt.bfloat16, isOutput=True)

    NS = G - G1  # groups 0..NS-1 are transposed on-chip

    with tile.TileContext(nc) as tc, ExitStack() as ctx:
        singles = ctx.enter_context(tc.tile_pool(name="singles", bufs=3))
        tn_pool = ctx.enter_context(tc.tile_pool(name="tn", bufs=4))
        tts_pool = ctx.enter_context(tc.tile_pool(name="tts", bufs=2))
        ttd_pool = ctx.enter_context(tc.tile_pool(name="ttd", bufs=max(G1, 1)))
        p_pool = ctx.enter_context(tc.tile_pool(name="pexp", bufs=3))
        pt_pool = ctx.enter_context(tc.tile_pool(name="ptT", bufs=2))
        o_pool = ctx.enter_context(tc.tile_pool(name="osb", bufs=1))
        sc_pool = ctx.enter_context(tc.tile_pool(name="scal", bufs=2))

        psum_s = ctx.enter_context(tc.tile_pool(name="psum_s", bufs=1, space="PSUM"))
        psum_tr = ctx.enter_context(tc.tile_pool(name="psum_tr", bufs=4, space="PSUM"))
        psum_pt = ctx.enter_context(tc.tile_pool(name="psum_pt", bufs=1, space="PSUM"))
        psum_o = ctx.enter_context(tc.tile_pool(name="psum_o", bufs=1, space="PSUM"))

        ident = singles.tile([128, 128], mybir.dt.bfloat16)

        # HAM warm-up: the clock-gate flip needs one full 3.4us window of
        # continuous normal-mode matmuls.  A DVE memset provides the dummy
        # operand ~1.3us before the gpsimd-built identity would be ready, so
        # the warmup (and hence the flip) starts as early as possible.
        dummy = singles.tile([128, 128], mybir.dt.bfloat16, name="dummy")
        nc.vector.memset(dummy, 0.0)
        warm = psum_s.tile([128, 512], mybir.dt.float32, name="s_ps")
        for w in range(34):
            nc.tensor.matmul(
                warm[:, (w % 4) * 128:(w % 4 + 1) * 128],
                lhsT=dummy,
                rhs=dummy,
                start=True,
                stop=True,
            )
        make_identity(nc, ident)

        # softmax denominators: col g for tr-groups (full exp accum), two
        # cols per shipped group's halves, last col for the final group's
        # DVE-reduced sum
        SUMW = NS + 2 * (G - 1 - NS) + 1
        sums_t = singles.tile([128, SUMW], mybir.dt.float32, name="sums_t")

        o_ab = [None]
        tn_tiles = {}   # pair -> tile [128, 8, BPC, D]
        ttd_tiles = {}  # g -> tile [128, BPC, DC, 512]
        tts_tiles = {}  # g -> tile [128, BPC, DC, 512] (on-chip transposed)
        p_tiles = {}    # g -> p_sb tile
        pt_tiles = {}   # g -> pt_sb tile
        mm1_psum = {}   # g -> s_ps tile

        def load_pair(p, eng, split=0):
            t = tn_pool.tile([128, 8, BPC, D], mybir.dt.bfloat16, name="tn_p")
            if split:
                step = 8 // split
                for h in range(split):
                    eng.dma_start(
                        out=t[:, h * step:(h + 1) * step],
                        in_=tn[p, :, h * step:(h + 1) * step],
                    )
            else:
                eng.dma_start(out=t, in_=tn[p])
            tn_tiles[p] = t

        def load_ttd(g, eng):
            gg = g - (G - G1)
            t = ttd_pool.tile([128, BPC, DC, 512], mybir.dt.bfloat16, name="tt_d")
            eng.dma_start(out=t, in_=tt[gg])
            ttd_tiles[g] = t

        def tr_slice(g, b, j, c0=0):
            """Transpose c-chunks c0..DC-1 of (group g, batch b, j-slice j):
            PE transposes into one PSUM tile, then one DVE copy into tts(g)."""
            if g not in tts_tiles:
                tts_tiles[g] = tts_pool.tile(
                    [128, BPC, DC, 512], mybir.dt.bfloat16, name="tts"
                )
            tn_p = tn_tiles[g // 2]
            jj0 = (g % 2) * 4
            tr = psum_tr.tile([128, DC, 128], mybir.dt.bfloat16, name="tr_ps")
            for c in range(c0, DC):
                nc.tensor.transpose(
                    tr[:, c - c0, :],
                    tn_p[:, jj0 + j, b, c * 128:(c + 1) * 128],
                    ident,
                )
            tr = tr[:, 0:DC - c0, :]
            dst = tts_tiles[g][:, b, c0:DC, j * 128:(j + 1) * 128]
            # copies split across DVE and ACT so the PSUM pool drains fast
            # enough for the 8-slice transpose burst; the last two slices'
            # copies stay on the faster DVE so mm1 never waits
            if (b, j) in ((1, 0), (1, 1), (1, 2)):
                nc.scalar.copy(dst, tr)
            else:
                nc.vector.tensor_copy(dst, tr)

        def mm1_chunk(g, c):
            """mm1 c-chunk, N=512, both batches col-tiled."""
            if g not in mm1_psum:
                mm1_psum[g] = psum_s.tile([128, 512], mybir.dt.float32, name="s_ps")
            s_ps = mm1_psum[g]
            src = ttd_tiles.get(g)
            if src is None:
                if NS - NH <= g < NS and c < HC:
                    for b in range(BPC):
                        nc.tensor.matmul(
                            s_ps[b * P:(b + 1) * P, :],
                            lhsT=qt_t[:, b, c, :],
                            rhs=tth_t[:, g - (NS - NH), b, c, :],
                            start=(c == 0),
                            stop=(c == DC - 1),
                            tile_position=(0, b * P),
                        )
                    return
                src = tts_tiles[g]
            for b in range(BPC):
                nc.tensor.matmul(
                    s_ps[b * P:(b + 1) * P, :],
                    lhsT=qt_t[:, b, c, :],
                    rhs=src[:, b, c, :],
                    start=(c == 0),
                    stop=(c == DC - 1),
                    tile_position=(0, b * P),
                )



        def mm1_half(g, c, h):
            """mm1 c-chunk restricted to n-column half h (N=256); used for
            shipped groups so exp of half h fires as soon as h stops.  h0
            targets the dedicated psum_sh bank (no buffer-reuse wait on the
            previous group's exp); h1 targets the regular s_ps bank."""
            if g not in mm1_psum:
                mm1_psum[g] = psum_s.tile([128, 512], mybir.dt.float32, name="s_ps")
            dst = mm1_psum[g][:, h * 256:(h + 1) * 256]
            src = ttd_tiles.get(g)
            if src is None:
                src = tts_tiles[g]
            for b in range(BPC):
                nc.tensor.matmul(
                    dst[b * P:(b + 1) * P, :],
                    lhsT=qt_t[:, b, c, :],
                    rhs=src[:, b, c, h * 256:(h + 1) * 256],
                    start=(c == 0),
                    stop=(c == DC - 1),
                    tile_position=(0, b * P),
                )

        def exp_full(g):
            s_ps = mm1_psum.pop(g)
            p_sb = p_pool.tile([128, 512], mybir.dt.bfloat16, name="p_sb")
            nc.scalar.activation(
                out=p_sb,
                in_=s_ps,
                func=mybir.ActivationFunctionType.Exp,
                scale=SCALE,
                accum_out=sums_t[:, g:g + 1],
            )
            p_tiles[g] = p_sb

        def exp_half(g, h):
            # per-half accum for shipped groups except the last (DVE reduce
            # in the tail instead)
            src = mm1_psum[g][:, h * 256:(h + 1) * 256]
            if g not in p_tiles:
                p_tiles[g] = p_pool.tile([128, 512], mybir.dt.bfloat16, name="p_sb")
            kw = {}
            if g < G - 1:
                col = NS + 2 * (g - NS) + h
                kw["accum_out"] = sums_t[:, col:col + 1]
            nc.scalar.activation(
                out=p_tiles[g][:, h * 256:(h + 1) * 256],
                in_=src,
                func=mybir.ActivationFunctionType.Exp,
                scale=SCALE,
                **kw,
            )

        def p_transpose(g, js=(0, 1, 2, 3)):
            p_sb = p_tiles[g]
            if g not in pt_tiles:
                pt_tiles[g] = pt_pool.tile([128, 4, 128], mybir.dt.bfloat16, name="pt_sb")
            pt_ps = psum_pt.tile([128, len(js), 128], mybir.dt.bfloat16, name="pt_ps")
            for k, j in enumerate(js):
                nc.tensor.transpose(
                    pt_ps[:, k, :], p_sb[:, j * 128:(j + 1) * 128], ident
                )
            nc.vector.tensor_copy(
                pt_tiles[g][:, js[0]:js[0] + len(js), :], pt_ps
            )

        def mm2_j(g, j):
            if o_ab[0] is None:
                o_a = psum_o.tile([128, 512], mybir.dt.float32, tag="o_a")
                o_b = psum_o.tile([128, 256], mybir.dt.float32, tag="o_b")
                o_ab[0] = (o_a, o_b)
            o_a, o_b = o_ab[0]
            tn_p = tn_tiles[g // 2]
            jj0 = (g % 2) * 4
            pt_sb = pt_tiles[g]
            nt = g * 4 + j
            for b in range(BPC):
                nc.tensor.matmul(
                    o_a[b * P:(b + 1) * P, :],
                    lhsT=pt_sb[:, j, b * P:(b + 1) * P],
                    rhs=tn_p[:, jj0 + j, b, 0:512],
                    start=(nt == 0),
                    stop=(nt == NT - 1),
                    tile_position=(0, b * P),
                )
                nc.tensor.matmul(
                    o_b[b * P:(b + 1) * P, :],
                    lhsT=pt_sb[:, j, b * P:(b + 1) * P],
                    rhs=tn_p[:, jj0 + j, b, 512:768],
                    start=(nt == 0),
                    stop=(nt == NT - 1),
                    tile_position=(0, b * P),
                )

        def finish():
            # sums_t[:, G-1] was filled by a DVE reduce over p(G-1)
            tot = sc_pool.tile([128, 1], mybir.dt.float32, name="tot")
            nc.vector.reduce_sum(tot, sums_t, axis=mybir.AxisListType.X)
            rec = sc_pool.tile([128, 1], mybir.dt.float32, name="rec")
            nc.vector.reciprocal(rec, tot)
            o_a, o_b = o_ab[0]
            o_sb = o_pool.tile([128, D], mybir.dt.bfloat16, name="o_sb")
            nc.vector.tensor_scalar_mul(o_sb[:, 0:512], o_a, rec)
            nc.scalar.mul(o_sb[:, 512:768], o_b, rec)
            nc.sync.dma_start(out=out[0], in_=o_sb[0:P, :])
            nc.scalar.dma_start(out=out[1], in_=o_sb[P:2 * P, :])

        # ---- DMA schedule ----
        # The 16-SDMA-engine pipe moves ~424 GB/s with a straggler engine a
        # few us behind; order transfers so each consumer runs just after its
        # semaphore: tt6 before pair3h1 (mm1(6) precedes mm2(6)), tt7 between
        # the pair3 halves.
        qt_t = singles.tile([128, BPC, DC, P], mybir.dt.bfloat16, name="qt_t")
        nc.scalar.dma_start(out=qt_t, in_=qt[:])
        load_pair(0, nc.sync, split=4)
        load_pair(1, nc.sync, split=2)
        load_pair(2, nc.sync, split=2)
        tth_t = singles.tile([128, NH, BPC, HC, 512], mybir.dt.bfloat16, name="tth_t")
        for hh in range(NH):
            nc.sync.dma_start(out=tth_t[:, hh], in_=tth[hh])
        load_ttd(G - 2, nc.sync)
        p3 = tn_pool.tile([128, 8, BPC, D], mybir.dt.bfloat16, name="tn_p")
        tn_tiles[3] = p3
        nc.sync.dma_start(out=p3[:, 0:4], in_=tn[3, :, 0:4])
        load_ttd(G - 1, nc.sync)
        nc.sync.dma_start(out=p3[:, 4:8], in_=tn[3, :, 4:8])

        # ---- phase-pipelined emission ----
        # Phase t (t < NS): transposes of group t, then mm1(t) (N=512
        # c-chunks), with pT(t-1)+mm2(t-1) slotted so every PE instruction's
        # off-engine producer (DVE copy / ACT exp) completes well before the
        # strict-FIFO tensor queue reaches it; exp(t) at the end.
        # Phase t in [NS, G): shipped group t - mm1 chunks first (no deps
        # beyond DMA), then pT/mm2 of t-1.
        # Phase G: split tail of group G-1, then finish.
        # Each steady phase is ONE transpose-mode mega-burst (48 token
        # transposes + the 4 pT transposes of the previous group) followed by
        # ONE normal-mode burst (mm2 of g-1, mm1 of g, exp) - mode switches
        # between transpose-mode and normal matmuls cost ~95ns each, so they
        # are minimized to two per phase.
        for t in range(G):
            g_pre = t - 1
            if t < NS:
                tc0 = HC if NS - NH <= t < NS else 0
                tr_slice(t, 0, 0, tc0)
                tr_slice(t, 1, 0, tc0)
                tr_slice(t, 0, 1, tc0)
                tr_slice(t, 1, 1, tc0)
                tr_slice(t, 0, 2, tc0)
                tr_slice(t, 1, 2, tc0)
                if g_pre >= 0:
                    p_transpose(g_pre)
                tr_slice(t, 0, 3, tc0)
                tr_slice(t, 1, 3, tc0)
                if g_pre >= 0:
                    mm2_j(g_pre, 0)
                    mm2_j(g_pre, 1)
                    mm2_j(g_pre, 2)
                    mm2_j(g_pre, 3)
                for c in range(DC):
                    mm1_chunk(t, c)
                exp_full(t)
            elif t < G - 1:
                # shipped group: mm1 in column halves so each exp half fires
                # as soon as its accumulation stops
                for c in range(DC):
                    mm1_half(t, c, 0)
                p_transpose(g_pre)
                exp_half(t, 0)
                for c in range(DC):
                    mm1_half(t, c, 1)
                mm2_j(g_pre, 0)
                mm2_j(g_pre, 1)
                mm2_j(g_pre, 2)
                exp_half(t, 1)
                mm2_j(g_pre, 3)
            else:
                # last group: its pT halves are prefetched into this phase so
                # the tail is a dense mm2 chain with no copy waits
                for c in range(DC):
                    mm1_half(t, c, 0)
                p_transpose(g_pre)
                exp_half(t, 0)
                for c in range(DC):
                    mm1_half(t, c, 1)
                mm2_j(g_pre, 0)
                p_transpose(t, js=(0, 1))
                mm2_j(g_pre, 1)
                mm2_j(g_pre, 2)
                exp_half(t, 1)
                mm2_j(g_pre, 3)

        # tail of the last group: pT(0,1) already copied; only pT(2,3) left
        g = G - 1
        mm2_j(g, 0)
        p_transpose(g, js=(2, 3))
        mm2_j(g, 1)
        # last group's row-sum: after the pt copies so the DVE queue never
        # delays mm2's pt dependency
        nc.vector.reduce_sum(
            sums_t[:, SUMW - 1:SUMW], p_tiles[g], axis=mybir.AxisListType.X
        )
        mm2_j(g, 2)
        mm2_j(g, 3)
        finish()

    nc.compile()
    return nc


def _get_nc() -> bass.Bass:
    global _cached_nc
    if _cached_nc is None:
        _cached_nc = build_bass_program()
    return _cached_nc


def _make_in_maps(x_latent: np.ndarray, prompts_latent: np.ndarray):
    x8 = np.ascontiguousarray(x_latent.astype(BF16)).reshape(N_CORES, BPC, N, D)
    q8 = prompts_latent.astype(BF16).reshape(N_CORES, BPC, P, D)
    # tn: [core, NPAIR, 128, 8, BPC, D] - j-major so split loads slice
    # contiguous per-partition runs
    tn_sw = np.ascontiguousarray(
        x8.reshape(N_CORES, BPC, NPAIR, 8, 128, D).transpose(0, 2, 4, 3, 1, 5)
    )
    # qt: [core, 128, BPC, DC, P]
    qt_sw = np.ascontiguousarray(
        q8.transpose(0, 1, 3, 2).reshape(N_CORES, BPC, DC, 128, P).transpose(0, 3, 1, 2, 4)
    )
    maps = []
    ttf = x8.transpose(0, 1, 3, 2)                      # [core, b, D, N]
    arr = ttf.reshape(N_CORES, BPC, DC, 128, G, 512)
    if G1 > 0:
        # tt: [core, G1, 128, BPC, DC, 512]
        tt_sw = np.ascontiguousarray(
            arr[:, :, :, :, G - G1:, :].transpose(0, 4, 3, 1, 2, 5)
        )
    # tth: first 3 c-chunks of groups G-G1-2, G-G1-1: [core, 2, 128, BPC, 3, 512]
    tth_sw = np.ascontiguousarray(
        arr[:, :, 0:3, :, G - G1 - NH:G - G1, :].transpose(0, 4, 3, 1, 2, 5)
    )
    for c in range(N_CORES):
        m = {"qt": qt_sw[c], "tn": tn_sw[c], "tth": tth_sw[c]}
        if G1 > 0:
            m["tt"] = tt_sw[c]
        maps.append(m)
    return maps


def run(x_latent: np.ndarray, prompts_latent: np.ndarray, trace: bool = False):
    """Run on all 8 cores; returns (output [16, 64, 768] f32, BassKernelResults)."""
    nc = _get_nc()
    in_maps = _make_in_maps(np.asarray(x_latent), np.asarray(prompts_latent))
    res = run_bass_kernel_spmd(nc, in_maps, list(range(N_CORES)), trace=trace)
    out = np.concatenate(
        [np.asarray(r["out"]).astype(np.float32) for r in res.results], axis=0
    )
    return out, res


def kernel(x_latent: np.ndarray, prompts_latent: np.ndarray) -> np.ndarray:
    out, _ = run(x_latent, prompts_latent, trace=False)
    return out
